# revision 2
# baseline (speedup 1.0000x reference)
"""Trainium2 Bass kernel: kNN-graph message passing block (MRConv + sync-BN + ReLU).

Math (per batch sample, matching the reference):
  xf (N, C) node features; dense kNN by squared L2 distance; K=16 (self included).
  maxrel = max_k xf[idx_k] - xf;  feat = interleave(xf, maxrel) (N, 2C)
  y = feat @ w.T (+b);  BN training-mode over (B, N) per channel; ReLU.

Distribution: one sample per NeuronCore (8 cores).  BN mean/var partial sums are
all-reduced across cores (768 floats).  b cancels inside BN and is ignored.

Per-core pipeline (channel-major layout throughout):
  1. PE: u[i, j] = xf_i . xf_j - 0.5*||xf_j||^2  (ranking-equivalent to -dist/2)
     via K-chunked matmuls with an appended ones-row (lhsT) / -0.5*x2-row (rhs).
  2. DVE: top-16 per row with max/max_index/match_replace (two top-8 rounds).
  3. PE: transpose+replicate indices in one matmul (out[p,s] = idx[s, p%16]).
  4. GPSIMD ap_gather: neighbor features, channels on partitions.
  5. DVE grouped max over K -> gmax.  (rel - x) is folded into the weights:
     y = (we-wo)^T.T @ x + wo^T.T @ gmax, accumulated in PSUM.
  6. ACT: PSUM->SBUF copies with accum_out giving per-channel sum; Square pass
     gives sum of squares; AllReduce; scale/shift; fused Relu apply; DMA out.
"""

import sys

import numpy as np

for _p in ("/opt/trn_rl_repo", "/root/.axon_site/_ro/trn_rl_repo"):
    if _p not in sys.path:
        sys.path.insert(0, _p)

import concourse.bass as bass
import concourse.mybir as mybir
import concourse.tile as tile
from concourse import bacc
from concourse.bass_utils import run_bass_kernel_spmd
from concourse.tile import add_dep_helper

B, C, OUT = 8, 192, 384
H = W = 56
N_FULL = H * W  # 3136
K = 16
EPS = 1e-5
NCORES = 8

F32 = mybir.dt.float32
I16 = mybir.dt.int16
U32 = mybir.dt.uint32
AF = mybir.ActivationFunctionType
ALU = mybir.AluOpType


def build(n=N_FULL, jt=448):
    """Build + compile the per-core program.  n must be a multiple of jt."""
    assert n % jt == 0
    nj = n // jt
    tot = float(B * n)

    nc = bacc.Bacc("TRN2", target_bir_lowering=False, debug=False)
    xT = nc.declare_dram_parameter("xT", [C, n], F32, isOutput=False)
    mx2 = nc.declare_dram_parameter("mx2", [1, n], F32, isOutput=False)
    wd = nc.declare_dram_parameter("wd", [C, OUT], F32, isOutput=False)
    wo = nc.declare_dram_parameter("wo", [C, OUT], F32, isOutput=False)
    ident = nc.declare_dram_parameter("ident", [128, 128], F32, isOutput=False)
    gamma = nc.declare_dram_parameter("gamma", [OUT], F32, isOutput=False)
    beta = nc.declare_dram_parameter("beta", [OUT], F32, isOutput=False)
    yout = nc.declare_dram_parameter("y", [OUT, n], F32, isOutput=True)

    bn_in = nc.dram_tensor("bn_in", [2 * OUT], F32)
    bn_out = nc.dram_tensor("bn_out", [2 * OUT], F32, addr_space="Shared")

    with tile.TileContext(nc) as tc:
        with tc.tile_pool(name="persist", bufs=1) as per:
            xc0 = per.tile([128, n], F32, tag="xc0")
            c1a = per.tile([65, n], F32, tag="c1a")
            c1b = per.tile([65, n], F32, tag="c1b")
            relc0 = per.tile([128, n], F32, tag="relc0")
            relc1 = per.tile([64, n], F32, tag="relc1")
            ypre = [per.tile([128, n], F32, tag=f"ypre{i}", name=f"ypre{i}") for i in range(3)]
            wd0 = per.tile([128, OUT], F32, tag="wd0")
            wd1 = per.tile([64, OUT], F32, tag="wd1")
            wo0 = per.tile([128, OUT], F32, tag="wo0")
            wo1 = per.tile([64, OUT], F32, tag="wo1")
            idt = per.tile([128, 128], F32, tag="idt")
            sums = [per.tile([128, nj], F32, tag=f"s1_{o}", name=f"s1_{o}") for o in range(3)]
            sqs = [per.tile([128, nj], F32, tag=f"s2_{o}", name=f"s2_{o}") for o in range(3)]
            epst = per.tile([128, 1], F32, tag="epst")

            nc.sync.dma_start(out=xc0, in_=xT[0:128, :])
            nc.sync.dma_start(out=c1a[0:64, :], in_=xT[128:192, :])
            nc.vector.memset(c1a[64:65, :], 1.0)
            nc.sync.dma_start(out=c1b[0:64, :], in_=xT[128:192, :])
            nc.sync.dma_start(out=c1b[64:65, :], in_=mx2[:, :])
            nc.sync.dma_start(out=wd0, in_=wd[0:128, :])
            nc.sync.dma_start(out=wd1, in_=wd[128:192, :])
            nc.sync.dma_start(out=wo0, in_=wo[0:128, :])
            nc.sync.dma_start(out=wo1, in_=wo[128:192, :])
            nc.sync.dma_start(out=idt, in_=ident[:, :])
            nc.vector.memset(epst, EPS)

            # ---- phase 1: per-row-block kNN + gather + grouped max ----
            with (
                tc.tile_pool(name="upool", bufs=2) as upool,
                tc.tile_pool(name="gpool", bufs=2) as gpool,
                tc.tile_pool(name="small", bufs=2) as small,
                tc.tile_pool(name="ups", bufs=4, space="PSUM") as ups,
                tc.tile_pool(name="repps", bufs=2, space="PSUM") as repps,
            ):
                for i0 in range(0, n, 128):
                    m = min(128, n - i0)
                    u = upool.tile([128, n], F32, tag="u")
                    for j in range(nj):
                        js = slice(j * jt, (j + 1) * jt)
                        ps = ups.tile([128, jt], F32, tag="ups")
                        nc.tensor.matmul(
                            out=ps[:m],
                            lhsT=xc0[:, i0 : i0 + m],
                            rhs=xc0[:, js],
                            start=True,
                            stop=False,
                        )
                        nc.tensor.matmul(
                            out=ps[:m],
                            lhsT=c1a[:, i0 : i0 + m],
                            rhs=c1b[:, js],
                            start=False,
                            stop=True,
                        )
                        nc.scalar.copy(out=u[:m, js], in_=ps[:m])

                    m1 = small.tile([128, 8], F32, tag="m1")
                    m2 = small.tile([128, 8], F32, tag="m2")
                    i1 = small.tile([128, 8], U32, tag="i1")
                    i2 = small.tile([128, 8], U32, tag="i2")
                    nc.vector.max(out=m1[:m], in_=u[:m])
                    nc.vector.max_index(out=i1[:m], in_max=m1[:m], in_values=u[:m])
                    nc.vector.match_replace(
                        out=u[:m], in_to_replace=m1[:m], in_values=u[:m],
                        imm_value=-1e30,
                    )
                    nc.vector.max(out=m2[:m], in_=u[:m])
                    nc.vector.max_index(out=i2[:m], in_max=m2[:m], in_values=u[:m])

                    idxf = small.tile([128, 16], F32, tag="idxf")
                    nc.vector.tensor_copy(out=idxf[:m, 0:8], in_=i1[:m])
                    nc.vector.tensor_copy(out=idxf[:m, 8:16], in_=i2[:m])
                    # columns replicated 8x: idxr[i, f] = idxf[i, f % 16]
                    idxr = small.tile([128, 128], F32, tag="idxr")
                    src = bass.AP(
                        tensor=idxf.tensor,
                        offset=idxf.offset,
                        ap=[[idxf.ap[0][0], m], [0, 8], [idxf.ap[-1][0], 16]],
                    )
                    nc.vector.tensor_copy(out=idxr[:m], in_=src)
                    # one matmul: rp[p, s] = idxf[s, p % 16]
                    rp = repps.tile([128, 128], F32, tag="rp")
                    nc.tensor.matmul(
                        out=rp[:, :m], lhsT=idxr[:m], rhs=idt[0:m, 0:m],
                        start=True, stop=True,
                    )
                    idxs = small.tile([128, 128], I16, tag="idxs")
                    nc.vector.tensor_copy(out=idxs[:, :m], in_=rp[:, :m])

                    g0 = gpool.tile([128, 128 * K], F32, tag="g0")
                    g1 = gpool.tile([64, 128 * K], F32, tag="g1")
                    nc.gpsimd.ap_gather(
                        out_ap=g0[:, : m * K], in_ap=xc0, idxs_ap=idxs[:, :m],
                        channels=128, num_elems=n, d=1, num_idxs=m * K,
                    )
                    nc.gpsimd.ap_gather(
                        out_ap=g1[:, : m * K], in_ap=c1a[0:64, :],
                        idxs_ap=idxs[0:64, :m],
                        channels=64, num_elems=n, d=1, num_idxs=m * K,
                    )
                    nc.vector.tensor_reduce(
                        out=relc0[:, i0 : i0 + m],
                        in_=g0[:, : m * K].rearrange("p (i k) -> p i k", k=K),
                        axis=mybir.AxisListType.X, op=ALU.max,
                    )
                    nc.vector.tensor_reduce(
                        out=relc1[:, i0 : i0 + m],
                        in_=g1[:, : m * K].rearrange("p (i k) -> p i k", k=K),
                        axis=mybir.AxisListType.X, op=ALU.max,
                    )

            # ---- phase 2: 1x1 conv + BN partials ----
            with (
                tc.tile_pool(name="fin", bufs=2) as fin,
                tc.tile_pool(name="yps", bufs=4, space="PSUM") as yps,
            ):
                for oc in range(3):
                    ocs = slice(oc * 128, (oc + 1) * 128)
                    for j in range(nj):
                        js = slice(j * jt, (j + 1) * jt)
                        ps = yps.tile([128, jt], F32, tag="yps")
                        nc.tensor.matmul(
                            out=ps, lhsT=wd0[:, ocs], rhs=xc0[:, js],
                            start=True, stop=False,
                        )
                        nc.tensor.matmul(
                            out=ps, lhsT=wd1[:, ocs], rhs=c1a[0:64, js],
                            start=False, stop=False,
                        )
                        nc.tensor.matmul(
                            out=ps, lhsT=wo0[:, ocs], rhs=relc0[:, js],
                            start=False, stop=False,
                        )
                        nc.tensor.matmul(
                            out=ps, lhsT=wo1[:, ocs], rhs=relc1[:, js],
                            start=False, stop=True,
                        )
                        nc.scalar.activation(
                            out=ypre[oc][:, js], in_=ps, func=AF.Copy,
                            accum_out=sums[oc][:, j : j + 1],
                        )
                        sq = fin.tile([128, jt], F32, tag="sqscr")
                        nc.scalar.activation(
                            out=sq, in_=ypre[oc][:, js], func=AF.Square,
                            accum_out=sqs[oc][:, j : j + 1],
                        )

                # ---- BN stats + sync ----
                stats = []
                for oc in range(3):
                    t1 = fin.tile([128, 1], F32, tag=f"t1_{oc}")
                    t2 = fin.tile([128, 1], F32, tag=f"t2_{oc}")
                    nc.vector.tensor_reduce(
                        out=t1, in_=sums[oc], axis=mybir.AxisListType.X, op=ALU.add
                    )
                    nc.vector.tensor_reduce(
                        out=t2, in_=sqs[oc], axis=mybir.AxisListType.X, op=ALU.add
                    )
                    nc.sync.dma_start(
                        out=bn_in[oc * 128 : (oc + 1) * 128].rearrange(
                            "(p a) -> p a", a=1
                        ),
                        in_=t1,
                    )
                    nc.sync.dma_start(
                        out=bn_in[OUT + oc * 128 : OUT + (oc + 1) * 128].rearrange(
                            "(p a) -> p a", a=1
                        ),
                        in_=t2,
                    )
                    stats.append((t1, t2))

                cc = nc.gpsimd.collective_compute(
                    "AllReduce",
                    ALU.add,
                    ins=[bn_in[:]],
                    outs=[bn_out[:]],
                    replica_groups=[list(range(NCORES))],
                )

                for oc in range(3):
                    ocs = slice(oc * 128, (oc + 1) * 128)
                    r1 = fin.tile([128, 1], F32, tag=f"r1_{oc}")
                    r2 = fin.tile([128, 1], F32, tag=f"r2_{oc}")
                    d1 = nc.sync.dma_start(
                        out=r1,
                        in_=bn_out[oc * 128 : (oc + 1) * 128].rearrange(
                            "(p a) -> p a", a=1
                        ),
                    )
                    d2 = nc.sync.dma_start(
                        out=r2,
                        in_=bn_out[OUT + oc * 128 : OUT + (oc + 1) * 128].rearrange(
                            "(p a) -> p a", a=1
                        ),
                    )
                    add_dep_helper(d1.ins, cc.ins, reason="allreduce before readback")
                    add_dep_helper(d2.ins, cc.ins, reason="allreduce before readback")

                    g_t = fin.tile([128, 1], F32, tag=f"g_{oc}")
                    b_t = fin.tile([128, 1], F32, tag=f"b_{oc}")
                    nc.sync.dma_start(
                        out=g_t, in_=gamma[ocs].rearrange("(p a) -> p a", a=1)
                    )
                    nc.sync.dma_start(
                        out=b_t, in_=beta[ocs].rearrange("(p a) -> p a", a=1)
                    )

                    mean = fin.tile([128, 1], F32, tag=f"mean_{oc}")
                    msq = fin.tile([128, 1], F32, tag=f"msq_{oc}")
                    var = fin.tile([128, 1], F32, tag=f"var_{oc}")
                    rstd = fin.tile([128, 1], F32, tag=f"rstd_{oc}")
                    scl = fin.tile([128, 1], F32, tag=f"scl_{oc}")
                    shf = fin.tile([128, 1], F32, tag=f"shf_{oc}")
                    nc.scalar.mul(out=mean, in_=r1, mul=1.0 / tot)
                    nc.scalar.mul(out=msq, in_=r2, mul=1.0 / tot)
                    nc.vector.tensor_mul(out=var, in0=mean, in1=mean)
                    nc.vector.tensor_sub(out=var, in0=msq, in1=var)
                    nc.scalar.activation(
                        out=rstd, in_=var, func=AF.Sqrt, bias=epst, scale=1.0
                    )
                    nc.vector.reciprocal(out=rstd, in_=rstd)
                    nc.vector.tensor_mul(out=scl, in0=g_t, in1=rstd)
                    nc.vector.tensor_mul(out=shf, in0=mean, in1=scl)
                    nc.vector.tensor_sub(out=shf, in0=b_t, in1=shf)

                    nc.scalar.activation(
                        out=ypre[oc], in_=ypre[oc], func=AF.Relu,
                        bias=shf, scale=scl,
                    )
                    nc.sync.dma_start(out=yout[ocs, :], in_=ypre[oc])

    nc.compile()
    return nc


_NC_CACHE = {}


def _get_nc():
    if "nc" not in _NC_CACHE:
        _NC_CACHE["nc"] = build()
    return _NC_CACHE["nc"]


def make_in_maps(x, w, gamma, beta, n=N_FULL):
    x = np.ascontiguousarray(np.asarray(x, np.float32))
    w = np.asarray(w, np.float32)
    nb = x.shape[0]
    xTf = x.reshape(nb, C, n)
    mx2 = -0.5 * np.sum(xTf * xTf, axis=1)  # (B, n)
    we = w[:, 0::2]
    woh = w[:, 1::2]
    wd_h = np.ascontiguousarray((we - woh).T)
    wo_h = np.ascontiguousarray(woh.T)
    identity = np.eye(128, dtype=np.float32)
    g = np.ascontiguousarray(np.asarray(gamma, np.float32))
    bt = np.ascontiguousarray(np.asarray(beta, np.float32))
    return [
        {
            "xT": np.ascontiguousarray(xTf[k]),
            "mx2": np.ascontiguousarray(mx2[k : k + 1]),
            "wd": wd_h,
            "wo": wo_h,
            "ident": identity,
            "gamma": g,
            "beta": bt,
        }
        for k in range(nb)
    ]


def kernel(x, w, b, gamma, beta):
    del b  # bias cancels inside training-mode BatchNorm
    nc = _get_nc()
    in_maps = make_in_maps(x, w, gamma, beta)
    res = run_bass_kernel_spmd(nc, in_maps, list(range(NCORES))).results
    y = np.stack([np.asarray(res[k]["y"]) for k in range(B)], axis=0)
    return y.reshape(B, OUT, H, W).astype(np.float32)


# revision 4
# speedup vs baseline: 2.7948x; 2.7948x over previous
"""Trainium2 Bass kernel: kNN-graph message passing block (MRConv + sync-BN + ReLU).

Math (per batch sample, matching the reference):
  xf (N, C) node features; dense kNN by squared L2 distance; K=16 (self included).
  maxrel = max_k xf[idx_k] - xf;  feat = interleave(xf, maxrel) (N, 2C)
  y = feat @ w.T (+b);  BN training-mode over (B, N) per channel; ReLU.

Distribution: one sample per NeuronCore (8 cores).  BN mean/var partial sums are
all-reduced across cores (768 floats).  b cancels inside BN and is ignored.

Per-core pipeline:
  1. PE: u[i, j] = xf_i . xf_j - 0.5*||xf_j||^2  (ranking-equivalent to -dist/2)
     via K-chunked matmuls with an appended ones-row (lhsT) / -0.5*x2-row (rhs).
  2. DVE: top-16 per row with max/max_index/match_replace (two top-8 rounds).
  3. Index shuffle into the SWDGE-gather wrapped layout entirely on-chip:
     B[j, k*8+ih] = idx[j, k] * (j//16 == ih), then one PE matmul with
     A8[j, p] = (j%16 == p%16) gives T[p, k*8+ih] = idx[ih*16 + p%16, k] —
     the per-k wrapped + core-replicated int16 index block for this row block.
  4. GPSIMD dma_gather (SWDGE) from a node-major DRAM copy of xf: 16 passes
     (one per neighbor slot) per node-chunk; DVE running max across passes.
  5. PE transposes put max_k(x_j) back channel-major.  (rel - x) is folded
     into the weights: y = (we-wo)^T.T @ x + wo^T.T @ gmax, PSUM-accumulated.
  6. ACT: PSUM->SBUF copies with accum_out giving per-channel sum; Square pass
     gives sum of squares; AllReduce; scale/shift; fused Relu apply; DMA out.
"""

import sys

import numpy as np

for _p in ("/opt/trn_rl_repo", "/root/.axon_site/_ro/trn_rl_repo"):
    if _p not in sys.path:
        sys.path.insert(0, _p)

import concourse.bass as bass
import concourse.mybir as mybir
import concourse.tile as tile
from concourse import bacc
from concourse.bass_utils import run_bass_kernel_spmd
from concourse.tile import add_dep_helper

B, C, OUT = 8, 192, 384
H = W = 56
N_FULL = H * W  # 3136
K = 16
EPS = 1e-5
NCORES = 8

F32 = mybir.dt.float32
I16 = mybir.dt.int16
U32 = mybir.dt.uint32
AF = mybir.ActivationFunctionType
ALU = mybir.AluOpType


def build(n=N_FULL, jt=448, rchunk=5):
    """Build + compile the per-core program.  n must be a multiple of jt."""
    assert n % jt == 0
    nj = n // jt
    tot = float(B * n)
    nblk = (n + 127) // 128  # 128-row blocks (last may be partial)
    npad = nblk * 128
    assert nblk % rchunk == 0
    nchunks = nblk // rchunk

    nc = bacc.Bacc("TRN2", target_bir_lowering=False, debug=False)
    xT = nc.declare_dram_parameter("xT", [C, n], F32, isOutput=False)
    xn = nc.declare_dram_parameter("xn", [npad, C], F32, isOutput=False)
    mx2 = nc.declare_dram_parameter("mx2", [1, n], F32, isOutput=False)
    wd = nc.declare_dram_parameter("wd", [C, OUT], F32, isOutput=False)
    wo = nc.declare_dram_parameter("wo", [C, OUT], F32, isOutput=False)
    ident = nc.declare_dram_parameter("ident", [128, 128], F32, isOutput=False)
    a8 = nc.declare_dram_parameter("a8", [128, 128], F32, isOutput=False)
    maskc = nc.declare_dram_parameter("maskc", [128, 8], F32, isOutput=False)
    gamma = nc.declare_dram_parameter("gamma", [OUT], F32, isOutput=False)
    beta = nc.declare_dram_parameter("beta", [OUT], F32, isOutput=False)
    yout = nc.declare_dram_parameter("y", [OUT, n], F32, isOutput=True)

    bn_in = nc.dram_tensor("bn_in", [2 * OUT], F32)
    bn_out = nc.dram_tensor("bn_out", [2 * OUT], F32, addr_space="Shared")

    with tile.TileContext(nc) as tc:
        with tc.tile_pool(name="persist", bufs=1) as per:
            xc0 = per.tile([128, n], F32, tag="xc0")
            c1a = per.tile([65, n], F32, tag="c1a")
            c1b = per.tile([65, n], F32, tag="c1b")
            relc0 = per.tile([128, n], F32, tag="relc0")
            relc1 = per.tile([64, n], F32, tag="relc1")
            ypre = [
                per.tile([128, n], F32, tag=f"ypre{i}", name=f"ypre{i}")
                for i in range(3)
            ]
            wd0 = per.tile([128, OUT], F32, tag="wd0")
            wd1 = per.tile([64, OUT], F32, tag="wd1")
            wo0 = per.tile([128, OUT], F32, tag="wo0")
            wo1 = per.tile([64, OUT], F32, tag="wo1")
            idt = per.tile([128, 128], F32, tag="idt")
            a8t = per.tile([128, 128], F32, tag="a8t")
            mskt = per.tile([128, 8], F32, tag="mskt")
            # per-chunk wrapped gather indices: [k, rb_in_chunk, ih] slots
            walls = [
                per.tile([128, K * rchunk * 8], I16, tag=f"wall{r}", name=f"wall{r}")
                for r in range(nchunks)
            ]
            sums = [
                per.tile([128, nj], F32, tag=f"s1_{o}", name=f"s1_{o}")
                for o in range(3)
            ]
            sqs = [
                per.tile([128, nj], F32, tag=f"s2_{o}", name=f"s2_{o}")
                for o in range(3)
            ]
            epst = per.tile([128, 1], F32, tag="epst")

            nc.sync.dma_start(out=xc0, in_=xT[0:128, :])
            nc.sync.dma_start(out=c1a[0:64, :], in_=xT[128:192, :])
            nc.vector.memset(c1a[64:65, :], 1.0)
            nc.sync.dma_start(out=c1b[0:64, :], in_=xT[128:192, :])
            nc.sync.dma_start(out=c1b[64:65, :], in_=mx2[:, :])
            nc.sync.dma_start(out=wd0, in_=wd[0:128, :])
            nc.sync.dma_start(out=wd1, in_=wd[128:192, :])
            nc.sync.dma_start(out=wo0, in_=wo[0:128, :])
            nc.sync.dma_start(out=wo1, in_=wo[128:192, :])
            nc.sync.dma_start(out=idt, in_=ident[:, :])
            nc.sync.dma_start(out=a8t, in_=a8[:, :])
            nc.sync.dma_start(out=mskt, in_=maskc[:, :])
            nc.vector.memset(epst, EPS)
            for wt in walls:
                nc.vector.memset(wt, 0)

            # ---- phase 1: per-row-block kNN top-16 + wrapped index build ----
            with (
                tc.tile_pool(name="upool", bufs=2) as upool,
                tc.tile_pool(name="small", bufs=2) as small,
                tc.tile_pool(name="ups", bufs=4, space="PSUM") as ups,
                tc.tile_pool(name="repps", bufs=2, space="PSUM") as repps,
            ):
                for rb in range(nblk):
                    i0 = rb * 128
                    m = min(128, n - i0)
                    nih = m // 16  # valid ih slots
                    u = upool.tile([128, n], F32, tag="u")
                    for j in range(nj):
                        js = slice(j * jt, (j + 1) * jt)
                        ps = ups.tile([128, jt], F32, tag="ups")
                        nc.tensor.matmul(
                            out=ps[:m],
                            lhsT=xc0[:, i0 : i0 + m],
                            rhs=xc0[:, js],
                            start=True,
                            stop=False,
                        )
                        nc.tensor.matmul(
                            out=ps[:m],
                            lhsT=c1a[:, i0 : i0 + m],
                            rhs=c1b[:, js],
                            start=False,
                            stop=True,
                        )
                        nc.scalar.copy(out=u[:m, js], in_=ps[:m])

                    m1 = small.tile([128, 8], F32, tag="m1")
                    m2 = small.tile([128, 8], F32, tag="m2")
                    i1 = small.tile([128, 8], U32, tag="i1")
                    i2 = small.tile([128, 8], U32, tag="i2")
                    nc.vector.max(out=m1[:m], in_=u[:m])
                    nc.vector.max_index(out=i1[:m], in_max=m1[:m], in_values=u[:m])
                    nc.vector.match_replace(
                        out=u[:m], in_to_replace=m1[:m], in_values=u[:m],
                        imm_value=-1e30,
                    )
                    nc.vector.max(out=m2[:m], in_=u[:m])
                    nc.vector.max_index(out=i2[:m], in_max=m2[:m], in_values=u[:m])

                    idxf = small.tile([128, 16], F32, tag="idxf")
                    nc.vector.tensor_copy(out=idxf[:m, 0:8], in_=i1[:m])
                    nc.vector.tensor_copy(out=idxf[:m, 8:16], in_=i2[:m])

                    # bmat[j, k*8+ih] = idx[j, k] * (j//16 == ih)
                    bmat = small.tile([128, 128], F32, tag="bmat")
                    idx_exp = bass.AP(
                        tensor=idxf.tensor,
                        offset=idxf.offset,
                        ap=[[idxf.ap[0][0], m], [idxf.ap[-1][0], K], [0, 8]],
                    )
                    msk_exp = bass.AP(
                        tensor=mskt.tensor,
                        offset=mskt.offset,
                        ap=[[mskt.ap[0][0], m], [0, K], [mskt.ap[-1][0], 8]],
                    )
                    nc.vector.tensor_mul(out=bmat[:m], in0=idx_exp, in1=msk_exp)
                    # tps[p, k*8+ih] = idx[ih*16 + p%16, k]
                    tps = repps.tile([128, 128], F32, tag="tps")
                    nc.tensor.matmul(
                        out=tps, lhsT=a8t[:m], rhs=bmat[:m], start=True, stop=True
                    )
                    # scatter into this chunk's wall: [k, rb_in_chunk, ih]
                    r = rb // rchunk
                    rloc = rb % rchunk
                    wt = walls[r]
                    dst = bass.AP(
                        tensor=wt.tensor,
                        offset=wt.offset + rloc * 8,
                        ap=[wt.ap[0], [rchunk * 8, K], [1, nih]],
                    )
                    srcap = bass.AP(
                        tensor=tps.tensor,
                        offset=tps.offset,
                        ap=[tps.ap[0], [8, K], [1, nih]],
                    )
                    nc.vector.tensor_copy(out=dst, in_=srcap)

            # ---- phase 2: chunked SWDGE gathers + running max + transpose ----
            with (
                tc.tile_pool(name="gpool", bufs=4) as gpool,
                tc.tile_pool(name="vpool", bufs=2) as vpool,
                tc.tile_pool(name="tps2", bufs=4, space="PSUM") as tpool,
            ):
                nidx = rchunk * 128
                for r in range(nchunks):
                    wt = walls[r]
                    vm = vpool.tile([128, rchunk, C], F32, tag="vm")
                    for k in range(K):
                        gk = gpool.tile([128, rchunk, C], F32, tag="gk")
                        nc.gpsimd.dma_gather(
                            gk,
                            xn[:, :],
                            wt[:, k * rchunk * 8 : (k + 1) * rchunk * 8],
                            num_idxs=nidx,
                            num_idxs_reg=nidx,
                            elem_size=C,
                        )
                        if k == 0:
                            first = gk
                        elif k == 1:
                            nc.vector.tensor_tensor(
                                out=vm, in0=first, in1=gk, op=ALU.max
                            )
                        else:
                            nc.vector.tensor_tensor(
                                out=vm, in0=vm, in1=gk, op=ALU.max
                            )
                    for g in range(rchunk):
                        gb = r * rchunk + g
                        node0 = gb * 128
                        valid = min(128, n - node0)
                        if valid <= 0:
                            continue
                        pt0 = tpool.tile([128, 128], F32, tag="pt0")
                        nc.tensor.transpose(
                            out=pt0, in_=vm[:, g, 0:128], identity=idt
                        )
                        nc.scalar.copy(
                            out=relc0[:, node0 : node0 + valid],
                            in_=pt0[:, 0:valid],
                        )
                        pt1 = tpool.tile([64, 128], F32, tag="pt1")
                        nc.tensor.transpose(
                            out=pt1, in_=vm[:, g, 128:192], identity=idt
                        )
                        nc.scalar.copy(
                            out=relc1[:, node0 : node0 + valid],
                            in_=pt1[:, 0:valid],
                        )

            # ---- phase 3: 1x1 conv + BN stats/sync + apply ----
            with (
                tc.tile_pool(name="fin", bufs=2) as fin,
                tc.tile_pool(name="yps", bufs=4, space="PSUM") as yps,
            ):
                for oc in range(3):
                    ocs = slice(oc * 128, (oc + 1) * 128)
                    for j in range(nj):
                        js = slice(j * jt, (j + 1) * jt)
                        ps = yps.tile([128, jt], F32, tag="yps")
                        nc.tensor.matmul(
                            out=ps, lhsT=wd0[:, ocs], rhs=xc0[:, js],
                            start=True, stop=False,
                        )
                        nc.tensor.matmul(
                            out=ps, lhsT=wd1[:, ocs], rhs=c1a[0:64, js],
                            start=False, stop=False,
                        )
                        nc.tensor.matmul(
                            out=ps, lhsT=wo0[:, ocs], rhs=relc0[:, js],
                            start=False, stop=False,
                        )
                        nc.tensor.matmul(
                            out=ps, lhsT=wo1[:, ocs], rhs=relc1[:, js],
                            start=False, stop=True,
                        )
                        nc.scalar.activation(
                            out=ypre[oc][:, js], in_=ps, func=AF.Copy,
                            accum_out=sums[oc][:, j : j + 1],
                        )
                        sq = fin.tile([128, jt], F32, tag="sqscr")
                        nc.scalar.activation(
                            out=sq, in_=ypre[oc][:, js], func=AF.Square,
                            accum_out=sqs[oc][:, j : j + 1],
                        )

                for oc in range(3):
                    t1 = fin.tile([128, 1], F32, tag=f"t1_{oc}", name=f"t1_{oc}")
                    t2 = fin.tile([128, 1], F32, tag=f"t2_{oc}", name=f"t2_{oc}")
                    nc.vector.tensor_reduce(
                        out=t1, in_=sums[oc], axis=mybir.AxisListType.X, op=ALU.add
                    )
                    nc.vector.tensor_reduce(
                        out=t2, in_=sqs[oc], axis=mybir.AxisListType.X, op=ALU.add
                    )
                    nc.sync.dma_start(
                        out=bn_in[oc * 128 : (oc + 1) * 128].rearrange(
                            "(p a) -> p a", a=1
                        ),
                        in_=t1,
                    )
                    nc.sync.dma_start(
                        out=bn_in[OUT + oc * 128 : OUT + (oc + 1) * 128].rearrange(
                            "(p a) -> p a", a=1
                        ),
                        in_=t2,
                    )

                cc = nc.gpsimd.collective_compute(
                    "AllReduce",
                    ALU.add,
                    ins=[bn_in[:]],
                    outs=[bn_out[:]],
                    replica_groups=[list(range(NCORES))],
                )

                for oc in range(3):
                    ocs = slice(oc * 128, (oc + 1) * 128)
                    r1 = fin.tile([128, 1], F32, tag=f"r1_{oc}", name=f"r1_{oc}")
                    r2 = fin.tile([128, 1], F32, tag=f"r2_{oc}", name=f"r2_{oc}")
                    d1 = nc.sync.dma_start(
                        out=r1,
                        in_=bn_out[oc * 128 : (oc + 1) * 128].rearrange(
                            "(p a) -> p a", a=1
                        ),
                    )
                    d2 = nc.sync.dma_start(
                        out=r2,
                        in_=bn_out[OUT + oc * 128 : OUT + (oc + 1) * 128].rearrange(
                            "(p a) -> p a", a=1
                        ),
                    )
                    add_dep_helper(d1.ins, cc.ins, reason="allreduce before readback")
                    add_dep_helper(d2.ins, cc.ins, reason="allreduce before readback")

                    g_t = fin.tile([128, 1], F32, tag=f"g_{oc}", name=f"g_{oc}")
                    b_t = fin.tile([128, 1], F32, tag=f"b_{oc}", name=f"b_{oc}")
                    nc.sync.dma_start(
                        out=g_t, in_=gamma[ocs].rearrange("(p a) -> p a", a=1)
                    )
                    nc.sync.dma_start(
                        out=b_t, in_=beta[ocs].rearrange("(p a) -> p a", a=1)
                    )

                    mean = fin.tile([128, 1], F32, tag=f"mean_{oc}", name=f"mean_{oc}")
                    msq = fin.tile([128, 1], F32, tag=f"msq_{oc}", name=f"msq_{oc}")
                    var = fin.tile([128, 1], F32, tag=f"var_{oc}", name=f"var_{oc}")
                    rstd = fin.tile([128, 1], F32, tag=f"rstd_{oc}", name=f"rstd_{oc}")
                    scl = fin.tile([128, 1], F32, tag=f"scl_{oc}", name=f"scl_{oc}")
                    shf = fin.tile([128, 1], F32, tag=f"shf_{oc}", name=f"shf_{oc}")
                    nc.scalar.mul(out=mean, in_=r1, mul=1.0 / tot)
                    nc.scalar.mul(out=msq, in_=r2, mul=1.0 / tot)
                    nc.vector.tensor_mul(out=var, in0=mean, in1=mean)
                    nc.vector.tensor_sub(out=var, in0=msq, in1=var)
                    nc.scalar.activation(
                        out=rstd, in_=var, func=AF.Sqrt, bias=epst, scale=1.0
                    )
                    nc.vector.reciprocal(out=rstd, in_=rstd)
                    nc.vector.tensor_mul(out=scl, in0=g_t, in1=rstd)
                    nc.vector.tensor_mul(out=shf, in0=mean, in1=scl)
                    nc.vector.tensor_sub(out=shf, in0=b_t, in1=shf)

                    nc.scalar.activation(
                        out=ypre[oc], in_=ypre[oc], func=AF.Relu,
                        bias=shf, scale=scl,
                    )
                    nc.sync.dma_start(out=yout[ocs, :], in_=ypre[oc])

    nc.compile()
    return nc


_NC_CACHE = {}


def _get_nc():
    if "nc" not in _NC_CACHE:
        _NC_CACHE["nc"] = build()
    return _NC_CACHE["nc"]


def make_in_maps(x, w, gamma, beta, n=N_FULL):
    x = np.ascontiguousarray(np.asarray(x, np.float32))
    w = np.asarray(w, np.float32)
    nb = x.shape[0]
    npad = ((n + 127) // 128) * 128
    xTf = x.reshape(nb, C, n)
    mx2 = -0.5 * np.sum(xTf * xTf, axis=1)  # (B, n)
    xnf = np.zeros((nb, npad, C), np.float32)
    xnf[:, :n, :] = xTf.transpose(0, 2, 1)
    we = w[:, 0::2]
    woh = w[:, 1::2]
    wd_h = np.ascontiguousarray((we - woh).T)
    wo_h = np.ascontiguousarray(woh.T)
    identity = np.eye(128, dtype=np.float32)
    jj = np.arange(128)
    a8_h = (jj[:, None] % 16 == jj[None, :] % 16).astype(np.float32)
    mask_h = (jj[:, None] // 16 == np.arange(8)[None, :]).astype(np.float32)
    g = np.ascontiguousarray(np.asarray(gamma, np.float32))
    bt = np.ascontiguousarray(np.asarray(beta, np.float32))
    return [
        {
            "xT": np.ascontiguousarray(xTf[k]),
            "xn": np.ascontiguousarray(xnf[k]),
            "mx2": np.ascontiguousarray(mx2[k : k + 1]),
            "wd": wd_h,
            "wo": wo_h,
            "ident": identity,
            "a8": a8_h,
            "maskc": mask_h,
            "gamma": g,
            "beta": bt,
        }
        for k in range(nb)
    ]


def kernel(x, w, b, gamma, beta):
    del b  # bias cancels inside training-mode BatchNorm
    nc = _get_nc()
    in_maps = make_in_maps(x, w, gamma, beta)
    res = run_bass_kernel_spmd(nc, in_maps, list(range(NCORES))).results
    y = np.stack([np.asarray(res[k]["y"]) for k in range(B)], axis=0)
    return y.reshape(B, OUT, H, W).astype(np.float32)


# revision 5
# speedup vs baseline: 3.9546x; 1.4150x over previous
"""Trainium2 Bass kernel: kNN-graph message passing block (MRConv + sync-BN + ReLU).

Math (per batch sample, matching the reference):
  xf (N, C) node features; dense kNN by squared L2 distance; K=16 (self included).
  maxrel = max_k xf[idx_k] - xf;  feat = interleave(xf, maxrel) (N, 2C)
  y = feat @ w.T (+b);  BN training-mode over (B, N) per channel; ReLU.

Distribution: one sample per NeuronCore (8 cores).  BN mean/var partial sums are
all-reduced across cores (768 floats).  b cancels inside BN and is ignored.

Per-core pipeline (all phases software-pipelined across 128-row node blocks):
  1. PE: u[i, j] = xf_i . xf_j - 0.5*||xf_j||^2  (ranking-equivalent to -dist/2)
     via K-chunked matmuls with an appended ones-row (lhsT) / -0.5*x2-row (rhs).
  2. DVE: top-16 per row with max/max_index/match_replace (two top-8 rounds).
  3. Index shuffle into the SWDGE-gather wrapped layout on-chip:
     B[j, k*8+ih] = idx[j, k] * (j//16 == ih), then one PE matmul with
     A8[j, p] = (j%16 == p%16) gives T[p, k*8+ih] = idx[ih*16 + p%16, k] —
     the per-k wrapped + core-replicated index block for this row block.
  4. GPSIMD dma_gather (SWDGE) from a node-major DRAM copy of xf, one pass per
     neighbor slot k=1..15 per 5-block node chunk (slot 0 is self: dist(i,i)=0
     ranks first, and max(x_i, .) is idempotent, so it is folded in at step 5).
     DVE running max across passes, spread 3 ops per row block to match the
     Q7 descriptor-generation rate without stalling the top-k stream.
  5. PE transposes put max_k(x_j) back channel-major; the final DVE max against
     x itself completes gmax.  (rel - x) is folded into the weights:
     y = (we-wo)^T.T @ x + wo^T.T @ gmax, PSUM-accumulated per 448-col tile.
  6. ACT: PSUM->SBUF copies with accum_out giving per-channel sum; Square pass
     gives sum of squares; AllReduce; scale/shift; fused Relu apply; DMA out.
"""

import sys
from collections import defaultdict

import numpy as np

for _p in ("/opt/trn_rl_repo", "/root/.axon_site/_ro/trn_rl_repo"):
    if _p not in sys.path:
        sys.path.insert(0, _p)

import concourse.bass as bass
import concourse.mybir as mybir
import concourse.tile as tile
from concourse import bacc
from concourse.bass_utils import run_bass_kernel_spmd
from concourse.tile import add_dep_helper

B, C, OUT = 8, 192, 384
H = W = 56
N_FULL = H * W  # 3136
K = 16
EPS = 1e-5
NCORES = 8

F32 = mybir.dt.float32
I16 = mybir.dt.int16
U32 = mybir.dt.uint32
AF = mybir.ActivationFunctionType
ALU = mybir.AluOpType


def build(n=N_FULL, jt=448, rchunk=5):
    """Build + compile the per-core program.  n must be a multiple of jt."""
    assert n % jt == 0
    nj = n // jt
    tot = float(B * n)
    nblk = (n + 127) // 128
    npad = nblk * 128
    assert nblk % rchunk == 0
    nchunks = nblk // rchunk

    # conv j-tile emitted with the last chunk covering its node range
    conv_after = defaultdict(list)
    for j in range(nj):
        c = ((j + 1) * jt - 1) // (rchunk * 128)
        conv_after[min(c, nchunks - 1)].append(j)

    nc = bacc.Bacc("TRN2", target_bir_lowering=False, debug=False)
    xT = nc.declare_dram_parameter("xT", [C, n], F32, isOutput=False)
    xn = nc.declare_dram_parameter("xn", [npad, C], F32, isOutput=False)
    mx2 = nc.declare_dram_parameter("mx2", [1, n], F32, isOutput=False)
    wd = nc.declare_dram_parameter("wd", [C, OUT], F32, isOutput=False)
    wo = nc.declare_dram_parameter("wo", [C, OUT], F32, isOutput=False)
    ident = nc.declare_dram_parameter("ident", [128, 128], F32, isOutput=False)
    a8 = nc.declare_dram_parameter("a8", [128, 128], F32, isOutput=False)
    maskc = nc.declare_dram_parameter("maskc", [128, 8], F32, isOutput=False)
    gamma = nc.declare_dram_parameter("gamma", [OUT], F32, isOutput=False)
    beta = nc.declare_dram_parameter("beta", [OUT], F32, isOutput=False)
    yout = nc.declare_dram_parameter("y", [OUT, n], F32, isOutput=True)

    bn_in = nc.dram_tensor("bn_in", [2 * OUT], F32)
    bn_out = nc.dram_tensor("bn_out", [2 * OUT], F32, addr_space="Shared")

    with tile.TileContext(nc) as tc:
        with (
            tc.tile_pool(name="persist", bufs=1) as per,
            tc.tile_pool(name="upool", bufs=2) as upool,
            tc.tile_pool(name="small", bufs=2) as small,
            tc.tile_pool(name="gpool", bufs=6) as gpool,
            tc.tile_pool(name="vpool", bufs=2) as vpool,
            tc.tile_pool(name="fin", bufs=2) as fin,
            tc.tile_pool(name="ups", bufs=2, space="PSUM") as ups,
            tc.tile_pool(name="tpsP", bufs=1, space="PSUM") as tpsP,
            tc.tile_pool(name="pt0P", bufs=2, space="PSUM") as pt0P,
            tc.tile_pool(name="pt1P", bufs=1, space="PSUM") as pt1P,
            tc.tile_pool(name="yps", bufs=2, space="PSUM") as yps,
        ):
            xc0 = per.tile([128, n], F32, tag="xc0")
            c1a = per.tile([65, n], F32, tag="c1a")
            c1b = per.tile([65, n], F32, tag="c1b")
            relc0 = per.tile([128, n], F32, tag="relc0")
            relc1 = per.tile([64, n], F32, tag="relc1")
            ypre = [
                per.tile([128, n], F32, tag=f"ypre{i}", name=f"ypre{i}")
                for i in range(3)
            ]
            wd0 = per.tile([128, OUT], F32, tag="wd0")
            wd1 = per.tile([64, OUT], F32, tag="wd1")
            wo0 = per.tile([128, OUT], F32, tag="wo0")
            wo1 = per.tile([64, OUT], F32, tag="wo1")
            idt = per.tile([128, 128], F32, tag="idt")
            a8t = per.tile([128, 128], F32, tag="a8t")
            mskt = per.tile([128, 8], F32, tag="mskt")
            walls = [
                per.tile([128, K * rchunk * 8], I16, tag=f"wall{r}", name=f"wall{r}")
                for r in range(nchunks)
            ]
            sums = [
                per.tile([128, nj], F32, tag=f"s1_{o}", name=f"s1_{o}")
                for o in range(3)
            ]
            sqs = [
                per.tile([128, nj], F32, tag=f"s2_{o}", name=f"s2_{o}")
                for o in range(3)
            ]
            epst = per.tile([128, 1], F32, tag="epst")

            nc.sync.dma_start(out=xc0, in_=xT[0:128, :])
            nc.sync.dma_start(out=c1a[0:64, :], in_=xT[128:192, :])
            nc.vector.memset(c1a[64:65, :], 1.0)
            nc.sync.dma_start(out=c1b[0:64, :], in_=xT[128:192, :])
            nc.sync.dma_start(out=c1b[64:65, :], in_=mx2[:, :])
            nc.sync.dma_start(out=wd0, in_=wd[0:128, :])
            nc.sync.dma_start(out=wd1, in_=wd[128:192, :])
            nc.sync.dma_start(out=wo0, in_=wo[0:128, :])
            nc.sync.dma_start(out=wo1, in_=wo[128:192, :])
            nc.sync.dma_start(out=idt, in_=ident[:, :])
            nc.sync.dma_start(out=a8t, in_=a8[:, :])
            nc.sync.dma_start(out=mskt, in_=maskc[:, :])
            nc.vector.memset(epst, EPS)
            for wt in walls:
                nc.vector.memset(wt, 0)

            gk_tiles = {}  # chunk -> [15 gather tiles]
            vm_tiles = {}  # chunk -> running-max tile

            def phase1(rb):
                i0 = rb * 128
                m = min(128, n - i0)
                nih = m // 16
                u = upool.tile([128, n], F32, tag="u", name="u")
                for j in range(nj):
                    js = slice(j * jt, (j + 1) * jt)
                    ps = ups.tile([128, jt], F32, tag="ups", name="ups")
                    nc.tensor.matmul(
                        out=ps[:m], lhsT=xc0[:, i0 : i0 + m], rhs=xc0[:, js],
                        start=True, stop=False,
                    )
                    nc.tensor.matmul(
                        out=ps[:m], lhsT=c1a[:, i0 : i0 + m], rhs=c1b[:, js],
                        start=False, stop=True,
                    )
                    nc.scalar.copy(out=u[:m, js], in_=ps[:m])

                m1 = small.tile([128, 8], F32, tag="m1", name="m1")
                m2 = small.tile([128, 8], F32, tag="m2", name="m2")
                i1 = small.tile([128, 8], U32, tag="i1", name="i1")
                i2 = small.tile([128, 8], U32, tag="i2", name="i2")
                nc.vector.max(out=m1[:m], in_=u[:m])
                nc.vector.max_index(out=i1[:m], in_max=m1[:m], in_values=u[:m])
                nc.vector.match_replace(
                    out=u[:m], in_to_replace=m1[:m], in_values=u[:m], imm_value=-1e30
                )
                nc.vector.max(out=m2[:m], in_=u[:m])
                nc.vector.max_index(out=i2[:m], in_max=m2[:m], in_values=u[:m])

                idxf = small.tile([128, 16], F32, tag="idxf", name="idxf")
                nc.vector.tensor_copy(out=idxf[:m, 0:8], in_=i1[:m])
                nc.vector.tensor_copy(out=idxf[:m, 8:16], in_=i2[:m])

                bmat = small.tile([128, 128], F32, tag="bmat", name="bmat")
                idx_exp = bass.AP(
                    tensor=idxf.tensor, offset=idxf.offset,
                    ap=[[idxf.ap[0][0], m], [1, K], [0, 8]],
                )
                msk_exp = bass.AP(
                    tensor=mskt.tensor, offset=mskt.offset,
                    ap=[[mskt.ap[0][0], m], [0, K], [1, 8]],
                )
                nc.vector.tensor_mul(out=bmat[:m], in0=idx_exp, in1=msk_exp)
                tps = tpsP.tile([128, 128], F32, tag="tps", name="tps")
                nc.tensor.matmul(
                    out=tps, lhsT=a8t[:m], rhs=bmat[:m], start=True, stop=True
                )
                r = rb // rchunk
                rloc = rb % rchunk
                wt = walls[r]
                dst = bass.AP(
                    tensor=wt.tensor, offset=wt.offset + rloc * 8,
                    ap=[wt.ap[0], [rchunk * 8, K], [1, nih]],
                )
                srcap = bass.AP(
                    tensor=tps.tensor, offset=tps.offset,
                    ap=[tps.ap[0], [8, K], [1, nih]],
                )
                nc.vector.tensor_copy(out=dst, in_=srcap)

            def emit_gathers(c):
                wt = walls[c]
                tiles = []
                for k in range(1, K):
                    gk = gpool.tile([128, rchunk, C], F32, tag="gk", name="gk")
                    nc.gpsimd.dma_gather(
                        gk, xn[:, :],
                        wt[:, k * rchunk * 8 : (k + 1) * rchunk * 8],
                        num_idxs=rchunk * 128, num_idxs_reg=rchunk * 128,
                        elem_size=C,
                    )
                    tiles.append(gk)
                gk_tiles[c] = tiles

            def emit_rmax(c, j):
                tiles = gk_tiles[c]
                if j == 0:
                    vm = vpool.tile([128, rchunk, C], F32, tag="vm", name="vm")
                    vm_tiles[c] = vm
                    nc.vector.tensor_tensor(
                        out=vm, in0=tiles[0], in1=tiles[1], op=ALU.max
                    )
                else:
                    vm = vm_tiles[c]
                    nc.vector.tensor_tensor(
                        out=vm, in0=vm, in1=tiles[j + 1], op=ALU.max
                    )

            def emit_fin(c):
                vm = vm_tiles[c]
                for g in range(rchunk):
                    gb = c * rchunk + g
                    node0 = gb * 128
                    valid = min(128, n - node0)
                    if valid <= 0:
                        continue
                    ns = slice(node0, node0 + valid)
                    pt0 = pt0P.tile([128, 128], F32, tag="pt0", name="pt0")
                    nc.tensor.transpose(out=pt0, in_=vm[:, g, 0:128], identity=idt)
                    nc.vector.tensor_tensor(
                        out=relc0[:, ns], in0=pt0[:, 0:valid], in1=xc0[:, ns],
                        op=ALU.max,
                    )
                    pt1 = pt1P.tile([64, 128], F32, tag="pt1", name="pt1")
                    nc.tensor.transpose(out=pt1, in_=vm[:, g, 128:192], identity=idt)
                    nc.vector.tensor_tensor(
                        out=relc1[:, ns], in0=pt1[:, 0:valid], in1=c1a[0:64, ns],
                        op=ALU.max,
                    )

            def emit_conv(j):
                js = slice(j * jt, (j + 1) * jt)
                for oc in range(3):
                    ocs = slice(oc * 128, (oc + 1) * 128)
                    ps = yps.tile([128, jt], F32, tag="yps", name="yps")
                    nc.tensor.matmul(
                        out=ps, lhsT=wd0[:, ocs], rhs=xc0[:, js],
                        start=True, stop=False,
                    )
                    nc.tensor.matmul(
                        out=ps, lhsT=wd1[:, ocs], rhs=c1a[0:64, js],
                        start=False, stop=False,
                    )
                    nc.tensor.matmul(
                        out=ps, lhsT=wo0[:, ocs], rhs=relc0[:, js],
                        start=False, stop=False,
                    )
                    nc.tensor.matmul(
                        out=ps, lhsT=wo1[:, ocs], rhs=relc1[:, js],
                        start=False, stop=True,
                    )
                    nc.scalar.activation(
                        out=ypre[oc][:, js], in_=ps, func=AF.Copy,
                        accum_out=sums[oc][:, j : j + 1],
                    )
                    sq = fin.tile([128, jt], F32, tag="sqscr", name="sqscr")
                    nc.scalar.activation(
                        out=sq, in_=ypre[oc][:, js], func=AF.Square,
                        accum_out=sqs[oc][:, j : j + 1],
                    )

            # ---- emission schedule: pipeline chunks against the top-k stream
            schedule = defaultdict(list)
            tail = []
            for c in range(nchunks):
                schedule[c * rchunk + rchunk - 1].append(("g", c))
                for j in range(K - 2):
                    rb = c * rchunk + rchunk + j // 3
                    item = ("r", c, j)
                    (schedule[rb] if rb < nblk else tail).append(item)
                rb = c * rchunk + rchunk + (K - 3) // 3
                fitem = ("f", c)
                (schedule[rb] if rb < nblk else tail).append(fitem)
                for j in conv_after[c]:
                    citem = ("conv", j)
                    (schedule[rb] if rb < nblk else tail).append(citem)

            def run_action(act):
                if act[0] == "g":
                    emit_gathers(act[1])
                elif act[0] == "r":
                    emit_rmax(act[1], act[2])
                elif act[0] == "f":
                    emit_fin(act[1])
                else:
                    emit_conv(act[1])

            for rb in range(nblk):
                phase1(rb)
                for act in schedule[rb]:
                    run_action(act)
            for act in tail:
                run_action(act)

            # ---- BN stats + sync + apply ----
            for oc in range(3):
                t1 = fin.tile([128, 1], F32, tag=f"t1_{oc}", name=f"t1_{oc}")
                t2 = fin.tile([128, 1], F32, tag=f"t2_{oc}", name=f"t2_{oc}")
                nc.vector.tensor_reduce(
                    out=t1, in_=sums[oc], axis=mybir.AxisListType.X, op=ALU.add
                )
                nc.vector.tensor_reduce(
                    out=t2, in_=sqs[oc], axis=mybir.AxisListType.X, op=ALU.add
                )
                nc.sync.dma_start(
                    out=bn_in[oc * 128 : (oc + 1) * 128].rearrange("(p a) -> p a", a=1),
                    in_=t1,
                )
                nc.sync.dma_start(
                    out=bn_in[OUT + oc * 128 : OUT + (oc + 1) * 128].rearrange(
                        "(p a) -> p a", a=1
                    ),
                    in_=t2,
                )

            cc = nc.gpsimd.collective_compute(
                "AllReduce",
                ALU.add,
                ins=[bn_in[:]],
                outs=[bn_out[:]],
                replica_groups=[list(range(NCORES))],
            )

            for oc in range(3):
                ocs = slice(oc * 128, (oc + 1) * 128)
                r1 = fin.tile([128, 1], F32, tag=f"r1_{oc}", name=f"r1_{oc}")
                r2 = fin.tile([128, 1], F32, tag=f"r2_{oc}", name=f"r2_{oc}")
                d1 = nc.sync.dma_start(
                    out=r1,
                    in_=bn_out[oc * 128 : (oc + 1) * 128].rearrange(
                        "(p a) -> p a", a=1
                    ),
                )
                d2 = nc.sync.dma_start(
                    out=r2,
                    in_=bn_out[OUT + oc * 128 : OUT + (oc + 1) * 128].rearrange(
                        "(p a) -> p a", a=1
                    ),
                )
                add_dep_helper(d1.ins, cc.ins, reason="allreduce before readback")
                add_dep_helper(d2.ins, cc.ins, reason="allreduce before readback")

                g_t = fin.tile([128, 1], F32, tag=f"g_{oc}", name=f"g_{oc}")
                b_t = fin.tile([128, 1], F32, tag=f"b_{oc}", name=f"b_{oc}")
                nc.sync.dma_start(
                    out=g_t, in_=gamma[ocs].rearrange("(p a) -> p a", a=1)
                )
                nc.sync.dma_start(
                    out=b_t, in_=beta[ocs].rearrange("(p a) -> p a", a=1)
                )

                mean = fin.tile([128, 1], F32, tag=f"mean_{oc}", name=f"mean_{oc}")
                msq = fin.tile([128, 1], F32, tag=f"msq_{oc}", name=f"msq_{oc}")
                var = fin.tile([128, 1], F32, tag=f"var_{oc}", name=f"var_{oc}")
                rstd = fin.tile([128, 1], F32, tag=f"rstd_{oc}", name=f"rstd_{oc}")
                scl = fin.tile([128, 1], F32, tag=f"scl_{oc}", name=f"scl_{oc}")
                shf = fin.tile([128, 1], F32, tag=f"shf_{oc}", name=f"shf_{oc}")
                nc.scalar.mul(out=mean, in_=r1, mul=1.0 / tot)
                nc.scalar.mul(out=msq, in_=r2, mul=1.0 / tot)
                nc.vector.tensor_mul(out=var, in0=mean, in1=mean)
                nc.vector.tensor_sub(out=var, in0=msq, in1=var)
                nc.scalar.activation(
                    out=rstd, in_=var, func=AF.Sqrt, bias=epst, scale=1.0
                )
                nc.vector.reciprocal(out=rstd, in_=rstd)
                nc.vector.tensor_mul(out=scl, in0=g_t, in1=rstd)
                nc.vector.tensor_mul(out=shf, in0=mean, in1=scl)
                nc.vector.tensor_sub(out=shf, in0=b_t, in1=shf)

                nc.scalar.activation(
                    out=ypre[oc], in_=ypre[oc], func=AF.Relu, bias=shf, scale=scl
                )
                nc.sync.dma_start(out=yout[ocs, :], in_=ypre[oc])

    nc.compile()
    return nc


_NC_CACHE = {}


def _get_nc():
    if "nc" not in _NC_CACHE:
        _NC_CACHE["nc"] = build()
    return _NC_CACHE["nc"]


def make_in_maps(x, w, gamma, beta, n=N_FULL):
    x = np.ascontiguousarray(np.asarray(x, np.float32))
    w = np.asarray(w, np.float32)
    nb = x.shape[0]
    npad = ((n + 127) // 128) * 128
    xTf = x.reshape(nb, C, n)
    mx2 = -0.5 * np.sum(xTf * xTf, axis=1)  # (B, n)
    xnf = np.zeros((nb, npad, C), np.float32)
    xnf[:, :n, :] = xTf.transpose(0, 2, 1)
    we = w[:, 0::2]
    woh = w[:, 1::2]
    wd_h = np.ascontiguousarray((we - woh).T)
    wo_h = np.ascontiguousarray(woh.T)
    identity = np.eye(128, dtype=np.float32)
    jj = np.arange(128)
    a8_h = (jj[:, None] % 16 == jj[None, :] % 16).astype(np.float32)
    mask_h = (jj[:, None] // 16 == np.arange(8)[None, :]).astype(np.float32)
    g = np.ascontiguousarray(np.asarray(gamma, np.float32))
    bt = np.ascontiguousarray(np.asarray(beta, np.float32))
    return [
        {
            "xT": np.ascontiguousarray(xTf[k]),
            "xn": np.ascontiguousarray(xnf[k]),
            "mx2": np.ascontiguousarray(mx2[k : k + 1]),
            "wd": wd_h,
            "wo": wo_h,
            "ident": identity,
            "a8": a8_h,
            "maskc": mask_h,
            "gamma": g,
            "beta": bt,
        }
        for k in range(nb)
    ]


def kernel(x, w, b, gamma, beta):
    del b  # bias cancels inside training-mode BatchNorm
    nc = _get_nc()
    in_maps = make_in_maps(x, w, gamma, beta)
    res = run_bass_kernel_spmd(nc, in_maps, list(range(NCORES))).results
    y = np.stack([np.asarray(res[k]["y"]) for k in range(B)], axis=0)
    return y.reshape(B, OUT, H, W).astype(np.float32)


# revision 6
# speedup vs baseline: 4.3108x; 1.0901x over previous
"""Trainium2 Bass kernel: kNN-graph message passing block (MRConv + sync-BN + ReLU).

Math (per batch sample, matching the reference):
  xf (N, C) node features; dense kNN by squared L2 distance; K=16 (self included).
  maxrel = max_k xf[idx_k] - xf;  feat = interleave(xf, maxrel) (N, 2C)
  y = feat @ w.T (+b);  BN training-mode over (B, N) per channel; ReLU.

Distribution: one sample per NeuronCore (8 cores).  BN mean/var partial sums are
all-reduced across cores (768 floats).  b cancels inside BN and is ignored.

Per-core pipeline (all phases software-pipelined across 128-row node blocks):
  1. PE: u[i, j] = xf_i . xf_j - 0.5*||xf_j||^2  (ranking-equivalent to -dist/2)
     via K-chunked matmuls with an appended ones-row (lhsT) / -0.5*x2-row (rhs).
  2. DVE: top-16 per row with max/max_index/match_replace (two top-8 rounds).
  3. Index shuffle into the SWDGE-gather wrapped layout on-chip:
     B[j, k*8+ih] = idx[j, k] * (j//16 == ih), then one PE matmul with
     A8[j, p] = (j%16 == p%16) gives T[p, k*8+ih] = idx[ih*16 + p%16, k] —
     the per-k wrapped + core-replicated index block for this row block.
  4. GPSIMD dma_gather (SWDGE) from a node-major DRAM copy of xf, one pass per
     neighbor slot k=1..15 per 5-block node chunk (slot 0 is self: dist(i,i)=0
     ranks first, and max(x_i, .) is idempotent, so it is folded in at step 5).
     DVE running max across passes, spread 3 ops per row block to match the
     Q7 descriptor-generation rate without stalling the top-k stream.
  5. PE transposes put max_k(x_j) back channel-major; the final DVE max against
     x itself completes gmax.  (rel - x) is folded into the weights:
     y = (we-wo)^T.T @ x + wo^T.T @ gmax, PSUM-accumulated per 448-col tile.
  6. ACT: PSUM->SBUF copies with accum_out giving per-channel sum; Square pass
     gives sum of squares; AllReduce; scale/shift; fused Relu apply; DMA out.
"""

import sys
from collections import defaultdict

import numpy as np

for _p in ("/opt/trn_rl_repo", "/root/.axon_site/_ro/trn_rl_repo"):
    if _p not in sys.path:
        sys.path.insert(0, _p)

import concourse.bass as bass
import concourse.mybir as mybir
import concourse.tile as tile
from concourse import bacc
from concourse.bass_utils import run_bass_kernel_spmd
from concourse.tile import add_dep_helper

B, C, OUT = 8, 192, 384
H = W = 56
N_FULL = H * W  # 3136
K = 16
EPS = 1e-5
NCORES = 8

F32 = mybir.dt.float32
I16 = mybir.dt.int16
U32 = mybir.dt.uint32
AF = mybir.ActivationFunctionType
ALU = mybir.AluOpType


def build(n=N_FULL, jt=448, chunks=None):
    """Build + compile the per-core program.  n must be a multiple of jt."""
    assert n % jt == 0
    nj = n // jt
    tot = float(B * n)
    nblk = (n + 127) // 128
    npad = nblk * 128
    if chunks is None:
        chunks = [5, 5, 5, 5, 3, 2] if nblk == 25 else [nblk]
    assert sum(chunks) == nblk
    nchunks = len(chunks)
    starts = [sum(chunks[:c]) for c in range(nchunks)]

    # conv j-tile emitted with the last chunk covering its node range
    conv_after = defaultdict(list)
    for j in range(nj):
        blocks_needed = -(-((j + 1) * jt) // 128)
        for c in range(nchunks):
            if starts[c] + chunks[c] >= blocks_needed:
                conv_after[c].append(j)
                break

    nc = bacc.Bacc("TRN2", target_bir_lowering=False, debug=False)
    xT = nc.declare_dram_parameter("xT", [C, n], F32, isOutput=False)
    xn = nc.declare_dram_parameter("xn", [npad, C], F32, isOutput=False)
    mx2 = nc.declare_dram_parameter("mx2", [1, n], F32, isOutput=False)
    wd = nc.declare_dram_parameter("wd", [C, OUT], F32, isOutput=False)
    wo = nc.declare_dram_parameter("wo", [C, OUT], F32, isOutput=False)
    ident = nc.declare_dram_parameter("ident", [128, 128], F32, isOutput=False)
    a8 = nc.declare_dram_parameter("a8", [128, 128], F32, isOutput=False)
    maskc = nc.declare_dram_parameter("maskc", [128, 8], F32, isOutput=False)
    gamma = nc.declare_dram_parameter("gamma", [OUT], F32, isOutput=False)
    beta = nc.declare_dram_parameter("beta", [OUT], F32, isOutput=False)
    yout = nc.declare_dram_parameter("y", [OUT, n], F32, isOutput=True)

    bn_in = nc.dram_tensor("bn_in", [2 * OUT], F32)
    bn_out = nc.dram_tensor("bn_out", [2 * OUT], F32, addr_space="Shared")

    with tile.TileContext(nc) as tc:
        with (
            tc.tile_pool(name="persist", bufs=1) as per,
            tc.tile_pool(name="upool", bufs=2) as upool,
            tc.tile_pool(name="small", bufs=2) as small,
            tc.tile_pool(name="gpool", bufs=6) as gpool,
            tc.tile_pool(name="vpool", bufs=2) as vpool,
            tc.tile_pool(name="fin", bufs=2) as fin,
            tc.tile_pool(name="ups", bufs=2, space="PSUM") as ups,
            tc.tile_pool(name="tpsP", bufs=1, space="PSUM") as tpsP,
            tc.tile_pool(name="pt0P", bufs=2, space="PSUM") as pt0P,
            tc.tile_pool(name="pt1P", bufs=1, space="PSUM") as pt1P,
            tc.tile_pool(name="yps", bufs=2, space="PSUM") as yps,
        ):
            xc0 = per.tile([128, n], F32, tag="xc0")
            c1a = per.tile([65, n], F32, tag="c1a")
            c1b = per.tile([65, n], F32, tag="c1b")
            relc0 = per.tile([128, n], F32, tag="relc0")
            relc1 = per.tile([64, n], F32, tag="relc1")
            ypre = [
                per.tile([128, n], F32, tag=f"ypre{i}", name=f"ypre{i}")
                for i in range(3)
            ]
            wd0 = per.tile([128, OUT], F32, tag="wd0")
            wd1 = per.tile([64, OUT], F32, tag="wd1")
            wo0 = per.tile([128, OUT], F32, tag="wo0")
            wo1 = per.tile([64, OUT], F32, tag="wo1")
            idt = per.tile([128, 128], F32, tag="idt")
            a8t = per.tile([128, 128], F32, tag="a8t")
            mskt = per.tile([128, 8], F32, tag="mskt")
            walls = [
                per.tile([128, K * chunks[r] * 8], I16, tag=f"wall{r}", name=f"wall{r}")
                for r in range(nchunks)
            ]
            sums = [
                per.tile([128, nj], F32, tag=f"s1_{o}", name=f"s1_{o}")
                for o in range(3)
            ]
            sqs = [
                per.tile([128, nj], F32, tag=f"s2_{o}", name=f"s2_{o}")
                for o in range(3)
            ]
            epst = per.tile([128, 1], F32, tag="epst")

            nc.sync.dma_start(out=xc0, in_=xT[0:128, :])
            nc.sync.dma_start(out=c1a[0:64, :], in_=xT[128:192, :])
            nc.vector.memset(c1a[64:65, :], 1.0)
            nc.sync.dma_start(out=c1b[0:64, :], in_=xT[128:192, :])
            nc.sync.dma_start(out=c1b[64:65, :], in_=mx2[:, :])
            nc.sync.dma_start(out=wd0, in_=wd[0:128, :])
            nc.sync.dma_start(out=wd1, in_=wd[128:192, :])
            nc.sync.dma_start(out=wo0, in_=wo[0:128, :])
            nc.sync.dma_start(out=wo1, in_=wo[128:192, :])
            nc.sync.dma_start(out=idt, in_=ident[:, :])
            nc.sync.dma_start(out=a8t, in_=a8[:, :])
            nc.sync.dma_start(out=mskt, in_=maskc[:, :])
            nc.vector.memset(epst, EPS)
            for wt in walls:
                nc.vector.memset(wt, 0)

            gk_tiles = {}  # chunk -> [15 gather tiles]
            vm_tiles = {}  # chunk -> running-max tile

            def phase1(rb):
                i0 = rb * 128
                m = min(128, n - i0)
                nih = m // 16
                u = upool.tile([128, n], F32, tag="u", name="u")
                for j in range(nj):
                    js = slice(j * jt, (j + 1) * jt)
                    ps = ups.tile([128, jt], F32, tag="ups", name="ups")
                    nc.tensor.matmul(
                        out=ps[:m], lhsT=xc0[:, i0 : i0 + m], rhs=xc0[:, js],
                        start=True, stop=False,
                    )
                    nc.tensor.matmul(
                        out=ps[:m], lhsT=c1a[:, i0 : i0 + m], rhs=c1b[:, js],
                        start=False, stop=True,
                    )
                    nc.scalar.copy(out=u[:m, js], in_=ps[:m])

                m1 = small.tile([128, 8], F32, tag="m1", name="m1")
                m2 = small.tile([128, 8], F32, tag="m2", name="m2")
                i1 = small.tile([128, 8], U32, tag="i1", name="i1")
                i2 = small.tile([128, 8], U32, tag="i2", name="i2")
                nc.vector.max(out=m1[:m], in_=u[:m])
                nc.vector.max_index(out=i1[:m], in_max=m1[:m], in_values=u[:m])
                nc.vector.match_replace(
                    out=u[:m], in_to_replace=m1[:m], in_values=u[:m], imm_value=-1e30
                )
                nc.vector.max(out=m2[:m], in_=u[:m])
                nc.vector.max_index(out=i2[:m], in_max=m2[:m], in_values=u[:m])

                idxf = small.tile([128, 16], F32, tag="idxf", name="idxf")
                nc.vector.tensor_copy(out=idxf[:m, 0:8], in_=i1[:m])
                nc.vector.tensor_copy(out=idxf[:m, 8:16], in_=i2[:m])

                bmat = small.tile([128, 128], F32, tag="bmat", name="bmat")
                idx_exp = bass.AP(
                    tensor=idxf.tensor, offset=idxf.offset,
                    ap=[[idxf.ap[0][0], m], [1, K], [0, 8]],
                )
                msk_exp = bass.AP(
                    tensor=mskt.tensor, offset=mskt.offset,
                    ap=[[mskt.ap[0][0], m], [0, K], [1, 8]],
                )
                nc.vector.tensor_mul(out=bmat[:m], in0=idx_exp, in1=msk_exp)
                tps = tpsP.tile([128, 128], F32, tag="tps", name="tps")
                nc.tensor.matmul(
                    out=tps, lhsT=a8t[:m], rhs=bmat[:m], start=True, stop=True
                )
                r = max(c for c in range(nchunks) if starts[c] <= rb)
                rloc = rb - starts[r]
                wt = walls[r]
                dst = bass.AP(
                    tensor=wt.tensor, offset=wt.offset + rloc * 8,
                    ap=[wt.ap[0], [chunks[r] * 8, K], [1, nih]],
                )
                srcap = bass.AP(
                    tensor=tps.tensor, offset=tps.offset,
                    ap=[tps.ap[0], [8, K], [1, nih]],
                )
                nc.vector.tensor_copy(out=dst, in_=srcap)

            def emit_gathers(c):
                wt = walls[c]
                rc = chunks[c]
                tiles = []
                for k in range(1, K):
                    gk = gpool.tile([128, rc, C], F32, tag="gk", name="gk")
                    nc.gpsimd.dma_gather(
                        gk, xn[:, :],
                        wt[:, k * rc * 8 : (k + 1) * rc * 8],
                        num_idxs=rc * 128, num_idxs_reg=rc * 128,
                        elem_size=C,
                    )
                    tiles.append(gk)
                gk_tiles[c] = tiles

            def emit_rmax(c, j):
                tiles = gk_tiles[c]
                if j == 0:
                    vm = vpool.tile([128, chunks[c], C], F32, tag="vm", name="vm")
                    vm_tiles[c] = vm
                    nc.vector.tensor_tensor(
                        out=vm, in0=tiles[0], in1=tiles[1], op=ALU.max
                    )
                else:
                    vm = vm_tiles[c]
                    nc.vector.tensor_tensor(
                        out=vm, in0=vm, in1=tiles[j + 1], op=ALU.max
                    )

            def emit_fin(c):
                vm = vm_tiles[c]
                for g in range(chunks[c]):
                    gb = starts[c] + g
                    node0 = gb * 128
                    valid = min(128, n - node0)
                    if valid <= 0:
                        continue
                    ns = slice(node0, node0 + valid)
                    pt0 = pt0P.tile([128, 128], F32, tag="pt0", name="pt0")
                    nc.tensor.transpose(out=pt0, in_=vm[:, g, 0:128], identity=idt)
                    nc.vector.tensor_tensor(
                        out=relc0[:, ns], in0=pt0[:, 0:valid], in1=xc0[:, ns],
                        op=ALU.max,
                    )
                    pt1 = pt1P.tile([64, 128], F32, tag="pt1", name="pt1")
                    nc.tensor.transpose(out=pt1, in_=vm[:, g, 128:192], identity=idt)
                    nc.vector.tensor_tensor(
                        out=relc1[:, ns], in0=pt1[:, 0:valid], in1=c1a[0:64, ns],
                        op=ALU.max,
                    )

            def emit_conv(j):
                js = slice(j * jt, (j + 1) * jt)
                for oc in range(3):
                    ocs = slice(oc * 128, (oc + 1) * 128)
                    ps = yps.tile([128, jt], F32, tag="yps", name="yps")
                    nc.tensor.matmul(
                        out=ps, lhsT=wd0[:, ocs], rhs=xc0[:, js],
                        start=True, stop=False,
                    )
                    nc.tensor.matmul(
                        out=ps, lhsT=wd1[:, ocs], rhs=c1a[0:64, js],
                        start=False, stop=False,
                    )
                    nc.tensor.matmul(
                        out=ps, lhsT=wo0[:, ocs], rhs=relc0[:, js],
                        start=False, stop=False,
                    )
                    nc.tensor.matmul(
                        out=ps, lhsT=wo1[:, ocs], rhs=relc1[:, js],
                        start=False, stop=True,
                    )
                    nc.scalar.activation(
                        out=ypre[oc][:, js], in_=ps, func=AF.Copy,
                        accum_out=sums[oc][:, j : j + 1],
                    )
                    sq = fin.tile([128, jt], F32, tag="sqscr", name="sqscr")
                    nc.scalar.activation(
                        out=sq, in_=ypre[oc][:, js], func=AF.Square,
                        accum_out=sqs[oc][:, j : j + 1],
                    )

            # ---- emission schedule: pipeline chunks against the top-k stream
            # consume pacing: one 640-idx gather costs ~1.12us/row-block of
            # Q7 time; a row block of top-k is ~18us.  Emit rmax op j only
            # after gather j+2 is expected complete so DVE never stalls.
            schedule = defaultdict(list)
            tail = []
            for c in range(nchunks):
                e_c = starts[c] + chunks[c] - 1
                schedule[e_c].append(("g", c))
                last_rb = e_c
                for j in range(K - 2):
                    off = 1 + int((j + 2) * 1.12 * chunks[c] / 18.0)
                    rb = e_c + off
                    last_rb = max(last_rb, rb)
                    item = ("r", c, j)
                    (schedule[rb] if rb < nblk else tail).append(item)
                fitem = ("f", c)
                (schedule[last_rb] if last_rb < nblk else tail).append(fitem)
                for j in conv_after[c]:
                    citem = ("conv", j)
                    (schedule[last_rb] if last_rb < nblk else tail).append(citem)

            def run_action(act):
                if act[0] == "g":
                    emit_gathers(act[1])
                elif act[0] == "r":
                    emit_rmax(act[1], act[2])
                elif act[0] == "f":
                    emit_fin(act[1])
                else:
                    emit_conv(act[1])

            for rb in range(nblk):
                phase1(rb)
                for act in schedule[rb]:
                    run_action(act)
            for act in tail:
                run_action(act)

            # ---- BN stats + sync + apply ----
            for oc in range(3):
                t1 = fin.tile([128, 1], F32, tag=f"t1_{oc}", name=f"t1_{oc}")
                t2 = fin.tile([128, 1], F32, tag=f"t2_{oc}", name=f"t2_{oc}")
                nc.vector.tensor_reduce(
                    out=t1, in_=sums[oc], axis=mybir.AxisListType.X, op=ALU.add
                )
                nc.vector.tensor_reduce(
                    out=t2, in_=sqs[oc], axis=mybir.AxisListType.X, op=ALU.add
                )
                nc.sync.dma_start(
                    out=bn_in[oc * 128 : (oc + 1) * 128].rearrange("(p a) -> p a", a=1),
                    in_=t1,
                )
                nc.sync.dma_start(
                    out=bn_in[OUT + oc * 128 : OUT + (oc + 1) * 128].rearrange(
                        "(p a) -> p a", a=1
                    ),
                    in_=t2,
                )

            cc = nc.gpsimd.collective_compute(
                "AllReduce",
                ALU.add,
                ins=[bn_in[:]],
                outs=[bn_out[:]],
                replica_groups=[list(range(NCORES))],
            )

            for oc in range(3):
                ocs = slice(oc * 128, (oc + 1) * 128)
                r1 = fin.tile([128, 1], F32, tag=f"r1_{oc}", name=f"r1_{oc}")
                r2 = fin.tile([128, 1], F32, tag=f"r2_{oc}", name=f"r2_{oc}")
                d1 = nc.sync.dma_start(
                    out=r1,
                    in_=bn_out[oc * 128 : (oc + 1) * 128].rearrange(
                        "(p a) -> p a", a=1
                    ),
                )
                d2 = nc.sync.dma_start(
                    out=r2,
                    in_=bn_out[OUT + oc * 128 : OUT + (oc + 1) * 128].rearrange(
                        "(p a) -> p a", a=1
                    ),
                )
                add_dep_helper(d1.ins, cc.ins, reason="allreduce before readback")
                add_dep_helper(d2.ins, cc.ins, reason="allreduce before readback")

                g_t = fin.tile([128, 1], F32, tag=f"g_{oc}", name=f"g_{oc}")
                b_t = fin.tile([128, 1], F32, tag=f"b_{oc}", name=f"b_{oc}")
                nc.sync.dma_start(
                    out=g_t, in_=gamma[ocs].rearrange("(p a) -> p a", a=1)
                )
                nc.sync.dma_start(
                    out=b_t, in_=beta[ocs].rearrange("(p a) -> p a", a=1)
                )

                mean = fin.tile([128, 1], F32, tag=f"mean_{oc}", name=f"mean_{oc}")
                msq = fin.tile([128, 1], F32, tag=f"msq_{oc}", name=f"msq_{oc}")
                var = fin.tile([128, 1], F32, tag=f"var_{oc}", name=f"var_{oc}")
                rstd = fin.tile([128, 1], F32, tag=f"rstd_{oc}", name=f"rstd_{oc}")
                scl = fin.tile([128, 1], F32, tag=f"scl_{oc}", name=f"scl_{oc}")
                shf = fin.tile([128, 1], F32, tag=f"shf_{oc}", name=f"shf_{oc}")
                nc.scalar.mul(out=mean, in_=r1, mul=1.0 / tot)
                nc.scalar.mul(out=msq, in_=r2, mul=1.0 / tot)
                nc.vector.tensor_mul(out=var, in0=mean, in1=mean)
                nc.vector.tensor_sub(out=var, in0=msq, in1=var)
                nc.scalar.activation(
                    out=rstd, in_=var, func=AF.Sqrt, bias=epst, scale=1.0
                )
                nc.vector.reciprocal(out=rstd, in_=rstd)
                nc.vector.tensor_mul(out=scl, in0=g_t, in1=rstd)
                nc.vector.tensor_mul(out=shf, in0=mean, in1=scl)
                nc.vector.tensor_sub(out=shf, in0=b_t, in1=shf)

                nc.scalar.activation(
                    out=ypre[oc], in_=ypre[oc], func=AF.Relu, bias=shf, scale=scl
                )
                nc.sync.dma_start(out=yout[ocs, :], in_=ypre[oc])

    nc.compile()
    return nc


_NC_CACHE = {}


def _get_nc():
    if "nc" not in _NC_CACHE:
        _NC_CACHE["nc"] = build()
    return _NC_CACHE["nc"]


def make_in_maps(x, w, gamma, beta, n=N_FULL):
    x = np.ascontiguousarray(np.asarray(x, np.float32))
    w = np.asarray(w, np.float32)
    nb = x.shape[0]
    npad = ((n + 127) // 128) * 128
    xTf = x.reshape(nb, C, n)
    mx2 = -0.5 * np.sum(xTf * xTf, axis=1)  # (B, n)
    xnf = np.zeros((nb, npad, C), np.float32)
    xnf[:, :n, :] = xTf.transpose(0, 2, 1)
    we = w[:, 0::2]
    woh = w[:, 1::2]
    wd_h = np.ascontiguousarray((we - woh).T)
    wo_h = np.ascontiguousarray(woh.T)
    identity = np.eye(128, dtype=np.float32)
    jj = np.arange(128)
    a8_h = (jj[:, None] % 16 == jj[None, :] % 16).astype(np.float32)
    mask_h = (jj[:, None] // 16 == np.arange(8)[None, :]).astype(np.float32)
    g = np.ascontiguousarray(np.asarray(gamma, np.float32))
    bt = np.ascontiguousarray(np.asarray(beta, np.float32))
    return [
        {
            "xT": np.ascontiguousarray(xTf[k]),
            "xn": np.ascontiguousarray(xnf[k]),
            "mx2": np.ascontiguousarray(mx2[k : k + 1]),
            "wd": wd_h,
            "wo": wo_h,
            "ident": identity,
            "a8": a8_h,
            "maskc": mask_h,
            "gamma": g,
            "beta": bt,
        }
        for k in range(nb)
    ]


def kernel(x, w, b, gamma, beta):
    del b  # bias cancels inside training-mode BatchNorm
    nc = _get_nc()
    in_maps = make_in_maps(x, w, gamma, beta)
    res = run_bass_kernel_spmd(nc, in_maps, list(range(NCORES))).results
    y = np.stack([np.asarray(res[k]["y"]) for k in range(B)], axis=0)
    return y.reshape(B, OUT, H, W).astype(np.float32)
